# revision 27
# baseline (speedup 1.0000x reference)
"""DistMult edge scoring on 8 Trainium2 NeuronCores.

score[e] = sum_d h[src[e],d] * fwd_rel[etype[e],d] * h[dst[e],d]

Strategy (edge-parallel): shard the 640k edges across 8 cores (80k each).
Per tile of T edges, each core issues three dma_gather calls (h[src],
h[dst], fwd_rel[etype]) spread across the 4 SWDGE queues — queue-parallel
descriptor generation is what makes the gathers run at fabric bandwidth
(~416 GB/s measured; a single queue is descgen-bound at ~8.7 ns/row).
fp32 rows (512 B) are gathered at line rate; fp16 rows (256 B) would hit
the SDMA read-modify-write penalty and go SLOWER, so everything stays fp32
(bit-exact reductions aside).

Dataflow per tile (edges-on-partitions, [128, T/128, 128] tiles):
  u *= w on DVE (in-place into the gathered u block)
  u *= v on DVE
  score[128, T/128] via ScalarE activation accumulate (per 128-col chunk),
  freeing DVE; DVE-reduce fallback available.

Index tensors are pre-marshalled on host into dma_gather's required wrap:
int16, index i at (partition i%16, column i//16), replicated 8x across the
128 partitions (one copy per GpSimd Q7 core). src/dst are concatenated per
tile into one [128, 2*T/16] tensor. The per-core edge count is padded to a
multiple of T with -1 (skipped by the gather); padded scores are dropped
on the host.

dma_gather calls use single_packet=False: single-packet mode hangs the
device above ~768 indices per call (HW-probed).
"""

import os
import sys

import numpy as np

sys.path.insert(0, "/opt/trn_rl_repo")

import concourse.bass as bass
import concourse.mybir as mybir
from concourse import bacc
from concourse.tile import TileContext

N_NODES = 10000
N_EDGES = 640000
D = 128
NUM_RELS = 500
N_CORES = 8
CORE_E = N_EDGES // N_CORES  # 80000

F32 = mybir.dt.float32
I16 = mybir.dt.int16


def build_program(
    core_e: int,
    tile_t: int,
    repeat: int = 1,
    bufs: int = 3,
    reduce_engine: str = "dve",
    mode: str = "full",  # "full" | "gather_only" | "compute_only"
    dma_scratch: int = 16384,
    gsplit: int = 1,  # split each gather into this many queue-spread pieces
):
    """Build the per-core Bass program (SPMD: same program, 8 cores).

    repeat>1 re-runs the main loop (same data) for marginal-cost timing.
    """
    assert tile_t % 128 == 0
    nt = -(-core_e // tile_t)  # tiles per core (last may be partial)
    t16 = tile_t // 16
    tcols = tile_t // 128

    nc = bacc.Bacc(num_swdge_queues=4, dynamic_dma_scratch_size=dma_scratch)
    h = nc.declare_dram_parameter("h", [N_NODES, D], F32, isOutput=False)
    rel = nc.declare_dram_parameter("fwd_rel", [NUM_RELS, D], F32, isOutput=False)
    isd = nc.declare_dram_parameter("idx_sd", [nt, 128, 2 * t16], I16, isOutput=False)
    iet = nc.declare_dram_parameter("idx_et", [nt, 128, t16], I16, isOutput=False)
    out = nc.declare_dram_parameter("scores", [nt, 128, tcols], F32, isOutput=True)

    with TileContext(nc) as tc:
        with (
            tc.tile_pool(name="gat", bufs=bufs) as gp,
            tc.tile_pool(name="idx", bufs=bufs) as ip,
            tc.tile_pool(name="res", bufs=bufs) as rp,
        ):
            if mode == "gather_fixed":
                # microbench-mimic: one preloaded idx tile, bare gathers
                ix0 = ip.tile([128, 2 * t16], I16, tag="ix0")
                nc.sync.dma_start(out=ix0[:], in_=isd[0])
                ixe0 = ip.tile([128, t16], I16, tag="ixe0")
                nc.sync.dma_start(out=ixe0[:], in_=iet[0])
                q = 0
                for _ in range(repeat):
                    for t in range(nt):
                        uv = gp.tile([128, 2 * tcols, D], F32, tag="uv")
                        w = gp.tile([128, tcols, D], F32, tag="w")
                        nc.gpsimd.dma_gather(
                            out_ap=uv[:, :tcols, :], in_ap=h[:], idxs_ap=ix0[:, :t16],
                            num_idxs=tile_t, num_idxs_reg=tile_t, elem_size=D,
                            single_packet=False, queue_num=q % 4,
                        )
                        nc.gpsimd.dma_gather(
                            out_ap=uv[:, tcols:, :], in_ap=h[:], idxs_ap=ix0[:, t16:],
                            num_idxs=tile_t, num_idxs_reg=tile_t, elem_size=D,
                            single_packet=False, queue_num=(q + 1) % 4,
                        )
                        nc.gpsimd.dma_gather(
                            out_ap=w[:], in_ap=h[:], idxs_ap=ix0[:, t16:],
                            num_idxs=tile_t, num_idxs_reg=tile_t, elem_size=D,
                            single_packet=False, queue_num=(q + 2) % 4,
                        )
                        q += 3
                s0 = rp.tile([128, tcols], F32, tag="s")
                nc.gpsimd.memset(s0[:], 0.0)
                for t in range(nt):
                    nc.sync.dma_start(out=out[t], in_=s0[:])
                nc.compile()
                return nc
            for _ in range(repeat):
                q = 0
                for t in range(nt):
                    valid = min(tile_t, core_e - t * tile_t)
                    ix_sd = ip.tile([128, 2 * t16], I16, tag="ixsd")
                    ix_w = ip.tile([128, t16], I16, tag="ixw")
                    nc.sync.dma_start(out=ix_sd[:], in_=isd[t])
                    nc.sync.dma_start(out=ix_w[:], in_=iet[t])

                    uv = gp.tile([128, 2 * tcols, D], F32, tag="uv")
                    w = gp.tile([128, tcols, D], F32, tag="w")
                    if valid < tile_t and mode != "compute_only":
                        # tail tile: -1 indices skip the write; zero-fill so
                        # the (discarded) padded scores stay finite
                        nc.gpsimd.memset(uv[:], 0.0)
                        nc.gpsimd.memset(w[:], 0.0)
                    if mode != "compute_only":
                        # (out block, table, idx slice) per logical gather,
                        # each optionally split gsplit ways across queues
                        streams = [
                            (uv[:, :tcols, :], h, ix_sd[:, :t16]),
                            (uv[:, tcols:, :], h, ix_sd[:, t16:]),
                            (w[:], rel, ix_w[:]),
                        ]
                        gs = gsplit
                        sub_t = tile_t // gs
                        sub_c = tcols // gs
                        sub_16 = t16 // gs
                        for dst, tab, ix in streams:
                            for k in range(gs):
                                sub_valid = max(
                                    0, min(sub_t, valid - k * sub_t)
                                )
                                if sub_valid == 0:
                                    continue  # fully padded; memset covers it
                                nc.gpsimd.dma_gather(
                                    out_ap=dst[:, k * sub_c : (k + 1) * sub_c, :],
                                    in_ap=tab[:],
                                    idxs_ap=ix[:, k * sub_16 : (k + 1) * sub_16],
                                    num_idxs=sub_t,
                                    num_idxs_reg=sub_valid,
                                    elem_size=D,
                                    single_packet=False,
                                    queue_num=q % 4,
                                )
                                q += 1

                    if mode == "gather_only":
                        continue
                    if mode == "compute_only":
                        nc.gpsimd.memset(uv[:], 0.5)
                        nc.gpsimd.memset(w[:], 0.5)

                    u = uv[:, :tcols, :]
                    v = uv[:, tcols:, :]
                    s = rp.tile([128, tcols], F32, tag="s")
                    if reduce_engine == "ttr":
                        # fused: u = u*w on DVE, then per 128-col chunk
                        # (u*v) with a running free-dim accumulate
                        nc.vector.tensor_mul(u, u, w[:])
                        for c in range(tcols):
                            nc.vector.tensor_tensor_reduce(
                                out=uv[:, c, :],
                                in0=uv[:, c, :],
                                in1=uv[:, tcols + c, :],
                                scale=1.0,
                                scalar=0.0,
                                op0=mybir.AluOpType.mult,
                                op1=mybir.AluOpType.add,
                                accum_out=s[:, c : c + 1],
                            )
                        nc.sync.dma_start(out=out[t], in_=s[:])
                        continue
                    nc.vector.tensor_mul(u, u, w[:])
                    nc.vector.tensor_mul(u, u, v)
                    if reduce_engine == "act":
                        for c in range(tcols):
                            nc.scalar.activation(
                                out=uv[:, c, :],
                                in_=uv[:, c, :],
                                func=mybir.ActivationFunctionType.Copy,
                                accum_out=s[:, c : c + 1],
                            )
                    else:
                        nc.vector.reduce_sum(s[:], u, axis=mybir.AxisListType.X)
                    nc.sync.dma_start(out=out[t], in_=s[:])

    nc.compile()
    return nc


def _wrap(ix_tile: np.ndarray) -> np.ndarray:
    """[T] int -> [128, T//16] int16 dma_gather wrap (8x replicated)."""
    t = ix_tile.shape[0]
    a = ix_tile.astype(np.int16).reshape(t // 16, 16).T
    return np.broadcast_to(a[None], (8, 16, t // 16)).reshape(128, t // 16)


def _pad(ix: np.ndarray, n: int) -> np.ndarray:
    if ix.shape[0] == n:
        return ix
    return np.concatenate([ix, np.full(n - ix.shape[0], -1, ix.dtype)])


def marshal_indices(src, dst, etype, nt, tile_t):
    """Build idx_sd [nt, 128, 2*T/16] (src block then dst block per tile)
    and idx_et [nt, 128, T/16], padding the tail tile with -1."""
    core_e = src.shape[0]
    isd = np.empty((nt, 128, 2 * tile_t // 16), np.int16)
    iet = np.empty((nt, 128, tile_t // 16), np.int16)
    for t in range(nt):
        sl = slice(t * tile_t, min((t + 1) * tile_t, core_e))
        s_t = _pad(src[sl], tile_t)
        d_t = _pad(dst[sl], tile_t)
        isd[t, :, : tile_t // 16] = _wrap(s_t)
        isd[t, :, tile_t // 16 :] = _wrap(d_t)
        iet[t] = _wrap(_pad(etype[sl], tile_t))
    return np.ascontiguousarray(isd), np.ascontiguousarray(iet)


_CACHE = {}
LAST_RESULTS = None  # test.py reads exec_time_ns from here when tracing

TILE_T = int(os.environ.get("KERNEL_TILE_T", "2048"))
BUFS = int(os.environ.get("KERNEL_BUFS", "8"))
REDUCE = os.environ.get("KERNEL_REDUCE", "dve")


def kernel(h, src, dst, etype, fwd_rel, rev_rel=None):
    global LAST_RESULTS
    from concourse.bass_utils import run_bass_kernel_spmd

    tile_t = TILE_T

    h = np.asarray(h, dtype=np.float32)
    fwd_rel = np.asarray(fwd_rel, dtype=np.float32)
    src = np.asarray(src)
    dst = np.asarray(dst)
    etype = np.asarray(etype)

    nt = -(-CORE_E // tile_t)
    key = (CORE_E, tile_t, BUFS, REDUCE)
    if key not in _CACHE:
        _CACHE[key] = build_program(CORE_E, tile_t, bufs=BUFS, reduce_engine=REDUCE)
    nc = _CACHE[key]

    in_maps = []
    for c in range(N_CORES):
        sl = slice(c * CORE_E, (c + 1) * CORE_E)
        isd, iet = marshal_indices(src[sl], dst[sl], etype[sl], nt, tile_t)
        in_maps.append({"h": h, "fwd_rel": fwd_rel, "idx_sd": isd, "idx_et": iet})

    trace = bool(os.environ.get("KERNEL_TRACE"))
    res = run_bass_kernel_spmd(
        nc, in_maps, core_ids=list(range(N_CORES)), trace=trace,
    )
    LAST_RESULTS = res

    outs = []
    for c in range(N_CORES):
        sw = res.results[c]["scores"]  # [nt, 128, tcols]
        outs.append(sw.transpose(0, 2, 1).reshape(nt * tile_t)[:CORE_E])
    return np.concatenate(outs).astype(np.float32)


# revision 34
# speedup vs baseline: 1.1434x; 1.1434x over previous
"""DistMult edge scoring on 8 Trainium2 NeuronCores.

score[e] = sum_d h[src[e],d] * fwd_rel[etype[e],d] * h[dst[e],d]

Strategy (edge-parallel): shard the 640k edges across 8 cores (80k each).
Per tile of T edges, each core issues three dma_gather calls (h[src],
h[dst], fwd_rel[etype]) spread across the 4 SWDGE queues — queue-parallel
descriptor generation is what makes the gathers run at fabric bandwidth
(~416 GB/s measured; a single queue is descgen-bound at ~8.7 ns/row).
fp32 rows (512 B) are gathered at line rate; fp16 rows (256 B) would hit
the SDMA read-modify-write penalty and go SLOWER, so everything stays fp32
(bit-exact reductions aside).

Dataflow per tile (edges-on-partitions, [128, T/128, 128] tiles):
  u *= w on DVE (in-place into the gathered u block)
  u *= v on DVE
  score[128, T/128] via ScalarE activation accumulate (per 128-col chunk),
  freeing DVE; DVE-reduce fallback available.

Index tensors are pre-marshalled on host into dma_gather's required wrap:
int16, index i at (partition i%16, column i//16), replicated 8x across the
128 partitions (one copy per GpSimd Q7 core). src/dst are concatenated per
tile into one [128, 2*T/16] tensor. The per-core edge count is padded to a
multiple of T with -1 (skipped by the gather); padded scores are dropped
on the host.

dma_gather calls use single_packet=False: single-packet mode hangs the
device above ~768 indices per call (HW-probed).
"""

import os
import sys

import numpy as np

sys.path.insert(0, "/opt/trn_rl_repo")

import concourse.bass as bass
import concourse.mybir as mybir
from concourse import bacc
from concourse.tile import TileContext

N_NODES = 10000
N_EDGES = 640000
D = 128
NUM_RELS = 500
N_CORES = 8
CORE_E = N_EDGES // N_CORES  # 80000

F32 = mybir.dt.float32
I16 = mybir.dt.int16


def build_program(
    core_e: int,
    tile_t: int,
    repeat: int = 1,
    bufs: int = 3,
    reduce_engine: str = "dve",
    mode: str = "full",  # "full" | "gather_only" | "compute_only"
    dma_scratch: int = 16384,
    # split each gather into this many queue-spread pieces. HW-probed:
    # gsplit=2 crashes the device (NRT_EXEC_UNIT_UNRECOVERABLE); keep 1.
    gsplit: int = 1,
    sep_tiles: bool = False,  # u/v in separate tiles instead of uv slices
    fused_io: bool = False,  # one idx DMA per tile + single final score store
):
    """Build the per-core Bass program (SPMD: same program, 8 cores).

    repeat>1 re-runs the main loop (same data) for marginal-cost timing.
    """
    assert tile_t % 128 == 0
    nt = -(-core_e // tile_t)  # tiles per core (last may be partial)
    t16 = tile_t // 16
    tcols = tile_t // 128

    nc = bacc.Bacc(num_swdge_queues=4, dynamic_dma_scratch_size=dma_scratch)
    h = nc.declare_dram_parameter("h", [N_NODES, D], F32, isOutput=False)
    rel = nc.declare_dram_parameter("fwd_rel", [NUM_RELS, D], F32, isOutput=False)
    if fused_io:
        isd = nc.declare_dram_parameter(
            "idx_sd", [nt, 128, 3 * t16], I16, isOutput=False
        )
        iet = None
        out = nc.declare_dram_parameter(
            "scores", [128, nt * tcols], F32, isOutput=True
        )
        return _build_fused(
            nc, h, rel, isd, out, core_e, tile_t, nt, t16, tcols, repeat, bufs
        )
    isd = nc.declare_dram_parameter("idx_sd", [nt, 128, 2 * t16], I16, isOutput=False)
    iet = nc.declare_dram_parameter("idx_et", [nt, 128, t16], I16, isOutput=False)
    out = nc.declare_dram_parameter("scores", [nt, 128, tcols], F32, isOutput=True)

    with TileContext(nc) as tc:
        with (
            tc.tile_pool(name="gat", bufs=bufs) as gp,
            tc.tile_pool(name="idx", bufs=bufs) as ip,
            tc.tile_pool(name="res", bufs=bufs) as rp,
        ):
            if mode == "gather_fixed":
                # microbench-mimic: one preloaded idx tile, bare gathers
                ix0 = ip.tile([128, 2 * t16], I16, tag="ix0")
                nc.sync.dma_start(out=ix0[:], in_=isd[0])
                ixe0 = ip.tile([128, t16], I16, tag="ixe0")
                nc.sync.dma_start(out=ixe0[:], in_=iet[0])
                q = 0
                for _ in range(repeat):
                    for t in range(nt):
                        uv = gp.tile([128, 2 * tcols, D], F32, tag="uv")
                        w = gp.tile([128, tcols, D], F32, tag="w")
                        nc.gpsimd.dma_gather(
                            out_ap=uv[:, :tcols, :], in_ap=h[:], idxs_ap=ix0[:, :t16],
                            num_idxs=tile_t, num_idxs_reg=tile_t, elem_size=D,
                            single_packet=False, queue_num=q % 4,
                        )
                        nc.gpsimd.dma_gather(
                            out_ap=uv[:, tcols:, :], in_ap=h[:], idxs_ap=ix0[:, t16:],
                            num_idxs=tile_t, num_idxs_reg=tile_t, elem_size=D,
                            single_packet=False, queue_num=(q + 1) % 4,
                        )
                        nc.gpsimd.dma_gather(
                            out_ap=w[:], in_ap=h[:], idxs_ap=ix0[:, t16:],
                            num_idxs=tile_t, num_idxs_reg=tile_t, elem_size=D,
                            single_packet=False, queue_num=(q + 2) % 4,
                        )
                        q += 3
                s0 = rp.tile([128, tcols], F32, tag="s")
                nc.gpsimd.memset(s0[:], 0.0)
                for t in range(nt):
                    nc.sync.dma_start(out=out[t], in_=s0[:])
                nc.compile()
                return nc
            for _ in range(repeat):
                q = 0
                for t in range(nt):
                    valid = min(tile_t, core_e - t * tile_t)
                    ix_sd = ip.tile([128, 2 * t16], I16, tag="ixsd")
                    ix_w = ip.tile([128, t16], I16, tag="ixw")
                    nc.sync.dma_start(out=ix_sd[:], in_=isd[t])
                    nc.sync.dma_start(out=ix_w[:], in_=iet[t])

                    if sep_tiles:
                        ut = gp.tile([128, tcols, D], F32, tag="ut")
                        vt = gp.tile([128, tcols, D], F32, tag="vt")
                        w = gp.tile([128, tcols, D], F32, tag="w")
                        if valid < tile_t and mode != "compute_only":
                            nc.gpsimd.memset(ut[:], 0.0)
                            nc.gpsimd.memset(vt[:], 0.0)
                            nc.gpsimd.memset(w[:], 0.0)
                        nc.gpsimd.dma_gather(
                            out_ap=ut[:], in_ap=h[:], idxs_ap=ix_sd[:, :t16],
                            num_idxs=tile_t, num_idxs_reg=valid, elem_size=D,
                            single_packet=False, queue_num=q % 4,
                        )
                        nc.gpsimd.dma_gather(
                            out_ap=vt[:], in_ap=h[:], idxs_ap=ix_sd[:, t16:],
                            num_idxs=tile_t, num_idxs_reg=valid, elem_size=D,
                            single_packet=False, queue_num=(q + 1) % 4,
                        )
                        nc.gpsimd.dma_gather(
                            out_ap=w[:], in_ap=rel[:], idxs_ap=ix_w[:],
                            num_idxs=tile_t, num_idxs_reg=valid, elem_size=D,
                            single_packet=False, queue_num=(q + 2) % 4,
                        )
                        q += 3
                        nc.vector.tensor_mul(ut[:], ut[:], w[:])
                        nc.vector.tensor_mul(ut[:], ut[:], vt[:])
                        s = rp.tile([128, tcols], F32, tag="s")
                        nc.vector.reduce_sum(s[:], ut[:], axis=mybir.AxisListType.X)
                        nc.sync.dma_start(out=out[t], in_=s[:])
                        continue
                    uv = gp.tile([128, 2 * tcols, D], F32, tag="uv")
                    w = gp.tile([128, tcols, D], F32, tag="w")
                    if valid < tile_t and mode != "compute_only":
                        # tail tile: -1 indices skip the write; zero-fill so
                        # the (discarded) padded scores stay finite
                        nc.gpsimd.memset(uv[:], 0.0)
                        nc.gpsimd.memset(w[:], 0.0)
                    if mode != "compute_only":
                        # (out block, table, idx slice) per logical gather,
                        # each optionally split gsplit ways across queues
                        streams = [
                            (uv[:, :tcols, :], h, ix_sd[:, :t16]),
                            (uv[:, tcols:, :], h, ix_sd[:, t16:]),
                            (w[:], rel, ix_w[:]),
                        ]
                        gs = gsplit
                        sub_t = tile_t // gs
                        sub_c = tcols // gs
                        sub_16 = t16 // gs
                        for dst, tab, ix in streams:
                            for k in range(gs):
                                sub_valid = max(
                                    0, min(sub_t, valid - k * sub_t)
                                )
                                if sub_valid == 0:
                                    continue  # fully padded; memset covers it
                                nc.gpsimd.dma_gather(
                                    out_ap=dst[:, k * sub_c : (k + 1) * sub_c, :],
                                    in_ap=tab[:],
                                    idxs_ap=ix[:, k * sub_16 : (k + 1) * sub_16],
                                    num_idxs=sub_t,
                                    num_idxs_reg=sub_valid,
                                    elem_size=D,
                                    single_packet=False,
                                    queue_num=q % 4,
                                )
                                q += 1

                    if mode == "gather_only":
                        continue
                    if mode == "compute_only":
                        nc.gpsimd.memset(uv[:], 0.5)
                        nc.gpsimd.memset(w[:], 0.5)

                    u = uv[:, :tcols, :]
                    v = uv[:, tcols:, :]
                    s = rp.tile([128, tcols], F32, tag="s")
                    if reduce_engine == "ttr":
                        # fused: u = u*w on DVE, then per 128-col chunk
                        # (u*v) with a running free-dim accumulate
                        nc.vector.tensor_mul(u, u, w[:])
                        for c in range(tcols):
                            nc.vector.tensor_tensor_reduce(
                                out=uv[:, c, :],
                                in0=uv[:, c, :],
                                in1=uv[:, tcols + c, :],
                                scale=1.0,
                                scalar=0.0,
                                op0=mybir.AluOpType.mult,
                                op1=mybir.AluOpType.add,
                                accum_out=s[:, c : c + 1],
                            )
                        nc.sync.dma_start(out=out[t], in_=s[:])
                        continue
                    nc.vector.tensor_mul(u, u, w[:])
                    nc.vector.tensor_mul(u, u, v)
                    if reduce_engine == "act":
                        for c in range(tcols):
                            nc.scalar.activation(
                                out=uv[:, c, :],
                                in_=uv[:, c, :],
                                func=mybir.ActivationFunctionType.Copy,
                                accum_out=s[:, c : c + 1],
                            )
                    else:
                        nc.vector.reduce_sum(s[:], u, axis=mybir.AxisListType.X)
                    nc.sync.dma_start(out=out[t], in_=s[:])

    nc.compile()
    return nc


def _build_fused(nc, h, rel, isd, out, core_e, tile_t, nt, t16, tcols, repeat, bufs):
    """One idx DMA per tile; scores accumulate in SBUF, single final store."""
    with TileContext(nc) as tc:
        with (
            tc.tile_pool(name="gat", bufs=bufs) as gp,
            tc.tile_pool(name="idx", bufs=bufs) as ip,
            tc.tile_pool(name="res", bufs=1) as rp,
        ):
            for _ in range(repeat):
                s_all = rp.tile([128, nt * tcols], F32, tag="sall")
                q = 0
                for t in range(nt):
                    valid = min(tile_t, core_e - t * tile_t)
                    ix = ip.tile([128, 3 * t16], I16, tag="ix")
                    nc.sync.dma_start(out=ix[:], in_=isd[t])

                    uv = gp.tile([128, 2 * tcols, D], F32, tag="uv")
                    w = gp.tile([128, tcols, D], F32, tag="w")
                    if valid < tile_t:
                        nc.gpsimd.memset(uv[:], 0.0)
                        nc.gpsimd.memset(w[:], 0.0)
                    nc.gpsimd.dma_gather(
                        out_ap=uv[:, :tcols, :], in_ap=h[:], idxs_ap=ix[:, :t16],
                        num_idxs=tile_t, num_idxs_reg=valid, elem_size=D,
                        single_packet=False, queue_num=q % 4,
                    )
                    nc.gpsimd.dma_gather(
                        out_ap=uv[:, tcols:, :], in_ap=h[:],
                        idxs_ap=ix[:, t16 : 2 * t16],
                        num_idxs=tile_t, num_idxs_reg=valid, elem_size=D,
                        single_packet=False, queue_num=(q + 1) % 4,
                    )
                    nc.gpsimd.dma_gather(
                        out_ap=w[:], in_ap=rel[:], idxs_ap=ix[:, 2 * t16 :],
                        num_idxs=tile_t, num_idxs_reg=valid, elem_size=D,
                        single_packet=False, queue_num=(q + 2) % 4,
                    )
                    q += 3

                    u = uv[:, :tcols, :]
                    v = uv[:, tcols:, :]
                    nc.vector.tensor_mul(u, u, w[:])
                    nc.vector.tensor_mul(u, u, v)
                    nc.vector.reduce_sum(
                        s_all[:, t * tcols : (t + 1) * tcols], u,
                        axis=mybir.AxisListType.X,
                    )
                nc.sync.dma_start(out=out[:], in_=s_all[:])
    nc.compile()
    return nc


def _wrap(ix_tile: np.ndarray) -> np.ndarray:
    """[T] int -> [128, T//16] int16 dma_gather wrap (8x replicated)."""
    t = ix_tile.shape[0]
    a = ix_tile.astype(np.int16).reshape(t // 16, 16).T
    return np.broadcast_to(a[None], (8, 16, t // 16)).reshape(128, t // 16)


def _pad(ix: np.ndarray, n: int) -> np.ndarray:
    if ix.shape[0] == n:
        return ix
    return np.concatenate([ix, np.full(n - ix.shape[0], -1, ix.dtype)])


def marshal_indices_fused(src, dst, etype, nt, tile_t):
    """[nt, 128, 3*T/16]: src, dst, etype wraps concatenated per tile."""
    core_e = src.shape[0]
    t16 = tile_t // 16
    ix = np.empty((nt, 128, 3 * t16), np.int16)
    for t in range(nt):
        sl = slice(t * tile_t, min((t + 1) * tile_t, core_e))
        ix[t, :, :t16] = _wrap(_pad(src[sl], tile_t))
        ix[t, :, t16 : 2 * t16] = _wrap(_pad(dst[sl], tile_t))
        ix[t, :, 2 * t16 :] = _wrap(_pad(etype[sl], tile_t))
    return np.ascontiguousarray(ix)


def marshal_indices(src, dst, etype, nt, tile_t):
    """Build idx_sd [nt, 128, 2*T/16] (src block then dst block per tile)
    and idx_et [nt, 128, T/16], padding the tail tile with -1."""
    core_e = src.shape[0]
    isd = np.empty((nt, 128, 2 * tile_t // 16), np.int16)
    iet = np.empty((nt, 128, tile_t // 16), np.int16)
    for t in range(nt):
        sl = slice(t * tile_t, min((t + 1) * tile_t, core_e))
        s_t = _pad(src[sl], tile_t)
        d_t = _pad(dst[sl], tile_t)
        isd[t, :, : tile_t // 16] = _wrap(s_t)
        isd[t, :, tile_t // 16 :] = _wrap(d_t)
        iet[t] = _wrap(_pad(etype[sl], tile_t))
    return np.ascontiguousarray(isd), np.ascontiguousarray(iet)


_CACHE = {}
LAST_RESULTS = None  # test.py reads exec_time_ns from here when tracing

TILE_T = int(os.environ.get("KERNEL_TILE_T", "2048"))
BUFS = int(os.environ.get("KERNEL_BUFS", "8"))
REDUCE = os.environ.get("KERNEL_REDUCE", "dve")


def kernel(h, src, dst, etype, fwd_rel, rev_rel=None):
    global LAST_RESULTS
    from concourse.bass_utils import run_bass_kernel_spmd

    tile_t = TILE_T

    h = np.asarray(h, dtype=np.float32)
    fwd_rel = np.asarray(fwd_rel, dtype=np.float32)
    src = np.asarray(src)
    dst = np.asarray(dst)
    etype = np.asarray(etype)

    nt = -(-CORE_E // tile_t)
    key = (CORE_E, tile_t, BUFS, REDUCE)
    if key not in _CACHE:
        _CACHE[key] = build_program(CORE_E, tile_t, bufs=BUFS, reduce_engine=REDUCE)
    nc = _CACHE[key]

    in_maps = []
    for c in range(N_CORES):
        sl = slice(c * CORE_E, (c + 1) * CORE_E)
        isd, iet = marshal_indices(src[sl], dst[sl], etype[sl], nt, tile_t)
        in_maps.append({"h": h, "fwd_rel": fwd_rel, "idx_sd": isd, "idx_et": iet})

    trace = bool(os.environ.get("KERNEL_TRACE"))
    res = run_bass_kernel_spmd(
        nc, in_maps, core_ids=list(range(N_CORES)), trace=trace,
    )
    LAST_RESULTS = res

    outs = []
    for c in range(N_CORES):
        sw = res.results[c]["scores"]  # [nt, 128, tcols]
        outs.append(sw.transpose(0, 2, 1).reshape(nt * tile_t)[:CORE_E])
    return np.concatenate(outs).astype(np.float32)
